# revision 2
# baseline (speedup 1.0000x reference)
"""StyleGAN2 up-2x blur (upfirdn2d, up=2, pad=(2,1), 4x4 kernel) on 8 trn2 cores.

x: (4, 64, 256, 256) f32, kernel: (4, 4) f32 -> out: (4, 64, 511, 511) f32.

Polyphase: out[2r+s, 2c+t] is a 2x2-tap conv of x with the flipped kernel
w = kernel[::-1, ::-1].  Sharding: data parallel over the 256 (N*C) planes,
32 planes/core.  Numerics: bf16 end-to-end (x rounded host-side, fp32 PSUM
accumulation, bf16 stores upconverted on host; a w_lo correction path doubles
the matmuls for kernels not exactly bf16-representable).  rel err ~5e-3.

Architecture (v2, rebuilt from microbenchmarks):
- STORES ARE HWDGE, NOT SWDGE.  A single HWDGE op with a 3-d AP
  [127 parts, 16 chunks, 2KB] spreads its descriptors across all 16 SDMA
  engines and sustains 433 GB/s aggregate (27 GB/s/engine); SWDGE topped out
  at ~210 GB/s (16KB descs, Q7 emission pacing).  Few big ops are key: each
  extra HWDGE op costs ~1-2us, so the 16.75MB output goes out in 4 ops of
  4.2MB (one per 4 plane-pairs).  2KB descriptors beat 4KB/16KB/131KB.
- DUAL-SHIFT MATMUL PACKING halves TensorE time vs 2-matmuls-per-psum:
  moving partitions = 64 input rows x {A = x[r, c], B = x[r, c-1]} column
  -shifted copies, so a single [128,128] stationary folds BOTH horizontal
  taps (via the A/B partition split) and BOTH vertical taps (banded diag)
  for BOTH s phases: one matmul pass per output element (128 matmuls of
  free=512 total, ~28us PE).  The two t phases use moving windows d=0/d=1
  over the same tile.
- The B copy is generated ON-CHIP by a partition-offset tensor_copy
  (SBUF[64:128] <- SBUF[0:64], works on vector/scalar/gpsimd), keeping HBM
  loads at 4.3MB.  Per-plane col-0 zeros are memset after the copy.
- gpsimd does no SWDGE descriptor emission anymore, so PSUM evacuation
  (f32->bf16 cast copies) round-robins over vector+scalar while gpsimd
  handles the dup copies; ~25us/engine, all under the DMA shadow.
- Per-chunk psum rows q=1..126 are complete; rows q=0/127 straddle 64-row
  input chunks and are computed separately for all planes via gather
  stationaries (seam rows {127,128,255,256,383,384} per plane).
- Loads (HWDGE, ~1KB descs) are split so pairs 0-1 land first and compute
  starts ~4us in; all DMA issues from the sync engine queue in
  load/store-interleaved order.
"""

import os
import numpy as np
import ml_dtypes

_BF = ml_dtypes.bfloat16
_NCORES = 8
_PL = 32            # planes per core
_NPAIR = _PL // 2   # plane pairs per core
_W = 256
_OW = 511

_cache = {}
last_exec_ns = None
last_results = None

_SEL = (63, 64, 127, 128, 191, 192)          # gather input rows per plane
_SEAM = (127, 128, 255, 256, 383, 384)       # seam output rows per plane


def _build(wlo_nz: bool):
    from contextlib import ExitStack
    import concourse.mybir as mybir
    import concourse.tile as tile
    from concourse import bacc

    BF = mybir.dt.bfloat16
    F32 = mybir.dt.float32

    nc = bacc.Bacc("TRN2", target_bir_lowering=False, debug=False)
    # A-copy input rows, host-packed: [pair, ch, r, 256*g + c] (+ zero pad col)
    xl = nc.dram_tensor("xl", [_NPAIR, 4, 64, 513], BF, kind="ExternalInput").ap()
    # seam gather rows: [unit, 6*pl + rk, c], rows _SEL of plane 16u+pl
    xs = nc.dram_tensor("xs", [2, 96, 256], BF, kind="ExternalInput").ap()
    sth = nc.dram_tensor("sth", [128, 2, 128], BF, kind="ExternalInput").ap()
    sgh = nc.dram_tensor("sgh", [96, 4, 96], BF, kind="ExternalInput").ap()
    if wlo_nz:
        stl = nc.dram_tensor("stl", [128, 2, 128], BF, kind="ExternalInput").ap()
        sgl = nc.dram_tensor("sgl", [96, 4, 96], BF, kind="ExternalInput").ap()
    # out[s, q, (mi, pp, g, ch, w)]: plane 8s+4mi+2pp+g, row 128*ch+q, col w
    out = nc.dram_tensor("out", [4, 127, 16384], BF, kind="ExternalOutput").ap()
    # outs[u, 6*pl + ok, w]: seam row _SEAM[ok] of plane 16u+pl
    outs = nc.dram_tensor("outs", [2, 96, 512], BF, kind="ExternalOutput").ap()

    nev = 0  # alternate evacuation copies between VectorE and ScalarE

    with tile.TileContext(nc) as tc, ExitStack() as ctx:
        cpool = ctx.enter_context(tc.tile_pool(name="const", bufs=1))
        tpool = ctx.enter_context(tc.tile_pool(name="tin", bufs=1))
        epool = ctx.enter_context(tc.tile_pool(name="edge", bufs=1))
        mpool = ctx.enter_context(tc.tile_pool(name="mega", bufs=1))
        bpool = ctx.enter_context(tc.tile_pool(name="bnd", bufs=1))
        ppool = ctx.enter_context(tc.tile_pool(name="ps", bufs=8, space="PSUM"))

        sth_t = cpool.tile([128, 2, 128], BF)
        nc.sync.dma_start(out=sth_t[:, :, :], in_=sth)
        sgh_t = cpool.tile([96, 4, 96], BF)
        nc.sync.dma_start(out=sgh_t[:, :, :], in_=sgh)
        if wlo_nz:
            stl_t = cpool.tile([128, 2, 128], BF)
            nc.sync.dma_start(out=stl_t[:, :, :], in_=stl)
            sgl_t = cpool.tile([96, 4, 96], BF)
            nc.sync.dma_start(out=sgl_t[:, :, :], in_=sgl)
        ets = []
        for u in range(2):
            e = epool.tile([96, 256], BF, tag=f"e{u}")
            nc.sync.dma_start(out=e[:, :], in_=xs[u])
            ets.append(e)

        # all input pairs resident: [128, pair, ch, 513]; A rows in parts 0:64
        T = tpool.tile([128, _NPAIR, 4, 513], BF)
        # out assembly: [q, s, mi, pp, g, ch, w] = 131KB/partition
        M = mpool.tile([128, 4, 2, 2, 2, 4, 512], BF)

        def load(a, b):  # load pairs a..b-1 (A rows)
            npc = (b - a) * 4
            nc.sync.dma_start(
                out=T[0:64, a:b, :, :].rearrange("r p c w -> r (p c) w"),
                in_=xl[a:b].rearrange("p c r w -> r (p c) w"))

        def copy_out(dst, src):
            nonlocal nev
            if nev % 2 == 0:
                nc.vector.tensor_copy(out=dst, in_=src)
            else:
                nc.scalar.copy(out=dst, in_=src)
            nev += 1

        def emit_pair(pi):
            # dup B = A shifted one col right (per 513-col chunk frame), then
            # zero the per-plane col-0 slots {0, 256}
            nc.gpsimd.tensor_copy(out=T[64:128, pi, :, 1:513],
                                  in_=T[0:64, pi, :, 0:512])
            nc.gpsimd.memset(T[64:128, pi, :, 0:512:256], 0.0)
            s_, mi, pp = pi // 4, (pi // 2) % 2, pi % 2
            for tt in (0, 1):
                for ch in range(4):
                    pt = ppool.tile([128, 512], F32, tag="ps")
                    mv = T[:, pi, ch, tt:tt + 512]
                    if wlo_nz:
                        nc.tensor.matmul(pt[:, :], sth_t[:, tt, :], mv,
                                         start=True, stop=False)
                        nc.tensor.matmul(pt[:, :], stl_t[:, tt, :], mv,
                                         start=False, stop=True)
                    else:
                        nc.tensor.matmul(pt[:, :], sth_t[:, tt, :], mv,
                                         start=True, stop=True)
                    # psum col 256*g + c -> plane g, out col 2c+tt
                    dst = M[0:127, s_, mi, pp, :, ch, tt:512:2]
                    copy_out(dst, pt[0:127, :].rearrange("q (g c) -> q g c", g=2))

        def emit_seam(u):
            pb = ppool.tile([96, 2, 256], F32, tag="ps")
            mms = []
            for tt, kx, mvw, pcw in ((0, 2, (0, 256), (0, 256)),
                                     (0, 0, (0, 255), (1, 256)),
                                     (1, 1, (0, 256), (0, 256)),
                                     (1, 3, (1, 256), (0, 255))):
                mms.append((tt, kx, mvw, pcw, "h"))
                if wlo_nz:
                    mms.append((tt, kx, mvw, pcw, "l"))
            for i, (tt, kx, mvw, pcw, wp) in enumerate(mms):
                sg = sgh_t if wp == "h" else sgl_t
                nc.tensor.matmul(
                    pb[:, tt, pcw[0]:pcw[1]], sg[:, kx, :],
                    ets[u][:, mvw[0]:mvw[1]],
                    start=(i == 0), stop=(i == len(mms) - 1))
            bt = bpool.tile([96, 512], BF, tag=f"bt{u}")
            copy_out(bt[:, 0:512:2], pb[:, 0, :])
            copy_out(bt[:, 1:512:2], pb[:, 1, :])
            nc.sync.dma_start(out=outs[u], in_=bt[:, :])

        def store(k):  # 4.2MB, [127 parts, 16 chunks, 2KB descs]
            nc.sync.dma_start(
                out=out[k].rearrange("q (c w) -> q c w", c=16),
                in_=M[0:127, k].rearrange("q a b g c w -> q (a b g c w)")
                .rearrange("q (c w) -> q c w", c=16))

        load(0, 2)
        load(2, 4)
        for pi in (0, 1, 2, 3):
            emit_pair(pi)
        load(4, 8)
        store(0)
        for pi in (4, 5, 6, 7):
            emit_pair(pi)
        load(8, 12)
        store(1)
        emit_seam(0)
        emit_seam(1)
        for pi in (8, 9, 10, 11):
            emit_pair(pi)
        load(12, 16)
        store(2)
        for pi in (12, 13, 14, 15):
            emit_pair(pi)
        store(3)

    nc.compile()
    return nc


def _host_arrays(w):
    w = np.asarray(w, np.float32)
    w_hi = w.astype(_BF).astype(np.float32)
    w_lo = w - w_hi
    wlo_nz = bool(np.any(w_lo != 0))

    def build_st(wv):
        # st[p=(a*64+r), t, q]; a=0: A copy (kx 2/3), a=1: B copy (kx 0/1)
        kxa = {(0, 0): 2, (0, 1): 0, (1, 0): 3, (1, 1): 1}
        st = np.zeros((2, 128, 128), np.float32)
        for t in range(2):
            for a in range(2):
                kx = kxa[(t, a)]
                for r in range(64):
                    p = a * 64 + r
                    st[t][p, 2 * r] += wv[2, kx]
                    if 2 * r + 2 <= 127:
                        st[t][p, 2 * r + 2] += wv[0, kx]
                    st[t][p, 2 * r + 1] += wv[1, kx]
                    if r >= 1:
                        st[t][p, 2 * r - 1] += wv[3, kx]
        return np.ascontiguousarray(st.transpose(1, 0, 2)).astype(_BF)

    def build_sg(wv):
        sg = np.zeros((96, 4, 96), np.float32)
        for pl in range(16):
            for ok in range(6):
                b = ok // 2
                taps = ([(2 * b, 1), (2 * b + 1, 3)] if ok % 2 == 0
                        else [(2 * b, 0), (2 * b + 1, 2)])
                for rk, ky in taps:
                    for kx in range(4):
                        sg[6 * pl + rk, kx, 6 * pl + ok] += wv[ky, kx]
        return np.ascontiguousarray(sg).astype(_BF)

    arrs = {"sth": build_st(w_hi), "sgh": build_sg(w_hi)}
    if wlo_nz:
        wlo_b = w_lo.astype(_BF).astype(np.float32)
        arrs["stl"] = build_st(wlo_b)
        arrs["sgl"] = build_sg(wlo_b)
    return wlo_nz, arrs


def kernel(x, kernel):
    global last_exec_ns, last_results
    from concourse.bass_utils import run_bass_kernel_spmd

    x = np.asarray(x, np.float32)
    w = np.asarray(kernel, np.float32)[::-1, ::-1]
    wlo_nz, warrs = _host_arrays(w)

    if wlo_nz not in _cache:
        _cache[wlo_nz] = _build(wlo_nz)
    nc = _cache[wlo_nz]

    planes = np.ascontiguousarray(x).reshape(256, 256, 256).astype(_BF)
    # xl[pair, ch, r, 256*g + c] = plane(2*pair+g)[64*ch + r, c]; col 512 = 0
    hi = planes.reshape(128, 2, 4, 64, 256)
    xl = np.zeros((128, 4, 64, 513), dtype=_BF)
    xl[..., :512] = hi.transpose(0, 2, 3, 1, 4).reshape(128, 4, 64, 512)
    # xs[u + 2*core, 6*pl + rk, c] = plane(32c+16u+pl)[_SEL[rk], c]
    xsa = planes[:, list(_SEL), :].reshape(_NCORES, 2, 96, 256)

    in_maps = []
    for c in range(_NCORES):
        mp = {"xl": xl[c * _NPAIR:(c + 1) * _NPAIR], "xs": xsa[c]}
        mp.update(warrs)
        in_maps.append(mp)

    trace = bool(os.environ.get("BLUR_TRACE"))
    tmpdir = os.environ.get("BLUR_TRACE_DIR") or None
    if trace:
        try:
            res = run_bass_kernel_spmd(nc, in_maps, list(range(_NCORES)),
                                       trace=True, tmpdir=tmpdir)
            last_exec_ns = res.exec_time_ns
        except Exception as e:
            print(f"trace run failed ({type(e).__name__}: {e}); retrying untraced")
            res = run_bass_kernel_spmd(nc, in_maps, list(range(_NCORES)))
            last_exec_ns = None
    else:
        res = run_bass_kernel_spmd(nc, in_maps, list(range(_NCORES)))
        last_exec_ns = None
    last_results = res

    full = np.empty((256, _OW, 512), dtype=_BF)
    for c in range(_NCORES):
        o = np.asarray(res.results[c]["out"]).reshape(4, 127, 2, 2, 2, 4, 512)
        sm = np.asarray(res.results[c]["outs"]).reshape(2, 16, 6, 512)
        # main rows: plane 8s+4mi+2pp+g, row 128*ch + q (q>=1, or q>=0 for ch 0)
        main = o.transpose(0, 2, 3, 4, 5, 1, 6).reshape(_PL, 4, 127, 512)
        blk = full[c * _PL:(c + 1) * _PL]
        blk[:, 0:127] = main[:, 0]
        blk[:, 129:255] = main[:, 1, 1:]
        blk[:, 257:383] = main[:, 2, 1:]
        blk[:, 385:511] = main[:, 3, 1:]
        for ok, row in enumerate(_SEAM):
            blk[:, row] = sm[:, :, ok].reshape(_PL, 512)
    return full[:, :, :_OW].reshape(4, 64, _OW, _OW).astype(np.float32)
